# revision 5
# baseline (speedup 1.0000x reference)
"""Causal self-attention (GQA + QK-RMSNorm + RoPE + q_gain) on 8 Trainium2 cores.

Sharding: 8 cores = 2 (batch) x 4 (KV head group).  Core c handles batch
c//4 and KV group g=c%4 (Q heads 4g..4g+3); it computes its heads'
attention and a partial output projection; the host sums 4 partials per
batch.

Precision strategy (rel-err budget 2e-2, measured ~1.4e-2):
- QKV projections: fp8e4m3 DoubleRow matmuls, 3-term residual scheme
  (w8 x8 + w8r x8 + w8 x8r) with weights pre-scaled x16 on the host so
  they sit in e4m3's normal range.  The x16 falls out of q/k via RMSNorm
  and is folded out of v via the Wproj scaling.
- Attention scores: bf16 (fp8 scores measured too lossy).
- Softmax: exp with a constant logit shift (EXPSHIFT) so e^logit fits
  e4m3.  Off-diagonal ("regular") blocks quantize p to fp8 and use
  DoubleRow for both the rowsum (paired ones) and PV (paired key blocks,
  v split hi+lo fp8); numerator and denominator use the SAME quantized p
  so the error acts as a small reweighting.  Diagonal blocks stay bf16
  (their early tokens have tiny row maxima that would flush in fp8).
- Output projection: bf16 (optionally fp8 3-term), y stored bf16.

All matmul cost on the PE is out-free-size bound; DoubleRow processes
2 contraction k-tiles per instruction at 0.5 cycles/column.
"""

import numpy as np

B, S, D = 2, 2048, 2048
H, KVH = 16, 4
HD = 128
G = H // KVH  # 4
NCORES = 8
ROPE_BASE = 10000.0
EPS = 1e-6

P = 128
SL = 512
NSL = S // SL        # 4
DK = D // P          # 16 contraction subtiles
NPAIR = DK // 2      # 8 DoubleRow pairs per term
SW = 16.0            # host weight scale for fp8 range
EXPSHIFT = 3.1       # logit shift so exp fits e4m3 (max logit ~8.15)

QKV_FP8 = True
PV_FP8 = True
PROJ_FP8 = False

_CACHE = {}


def _build_program():
    from contextlib import ExitStack

    import concourse.bass as bass
    import concourse.tile as tile
    from concourse import bacc, mybir

    f32 = mybir.dt.float32
    bf = mybir.dt.bfloat16
    f8 = mybir.dt.float8e4
    AF = mybir.ActivationFunctionType
    OP = mybir.AluOpType
    DR = mybir.MatmulPerfMode.DoubleRow

    nc = bacc.Bacc("TRN2", target_bir_lowering=False)

    wdt = f8 if QKV_FP8 else bf
    x8_d = nc.dram_tensor("x8", [D, S], wdt, kind="ExternalInput").ap()
    wq8_d = nc.dram_tensor("wq8", [D, G * HD], wdt, kind="ExternalInput").ap()
    wk8_d = nc.dram_tensor("wk8", [D, HD], wdt, kind="ExternalInput").ap()
    wv8_d = nc.dram_tensor("wv8", [D, HD], wdt, kind="ExternalInput").ap()
    if QKV_FP8:
        x8r_d = nc.dram_tensor("x8r", [D, S], f8, kind="ExternalInput").ap()
        wq8r_d = nc.dram_tensor("wq8r", [D, G * HD], f8, kind="ExternalInput").ap()
        wk8r_d = nc.dram_tensor("wk8r", [D, HD], f8, kind="ExternalInput").ap()
        wv8r_d = nc.dram_tensor("wv8r", [D, HD], f8, kind="ExternalInput").ap()
    pdt = f8 if PROJ_FP8 else bf
    wp_d = nc.dram_tensor("wp", [G * HD, D], pdt, kind="ExternalInput").ap()
    if PROJ_FP8:
        wpr_d = nc.dram_tensor("wpr", [G * HD, D], f8, kind="ExternalInput").ap()
    cosT_d = nc.dram_tensor("cosT", [HD, S], bf, kind="ExternalInput").ap()
    sinT_d = nc.dram_tensor("sinT", [HD, S], bf, kind="ExternalInput").ap()
    jT_d = nc.dram_tensor("jT", [HD, HD], bf, kind="ExternalInput").ap()
    qgain_d = nc.dram_tensor("qgain", [1, G], f32, kind="ExternalInput").ap()
    y_d = nc.dram_tensor("y", [S, D], bf, kind="ExternalOutput").ap()

    x3 = x8_d.rearrange("(o p) s -> p o s", p=P)
    if QKV_FP8:
        x3r = x8r_d.rearrange("(o p) s -> p o s", p=P)
    wq3 = wq8_d.rearrange("(o p) m -> p o m", p=P)
    wk3 = wk8_d.rearrange("(o p) m -> p o m", p=P)
    wv3 = wv8_d.rearrange("(o p) m -> p o m", p=P)

    with tile.TileContext(nc) as tc, ExitStack() as top:
        res = top.enter_context(tc.tile_pool(name="resident", bufs=1))

        # ---- small constants ----
        ones_f = res.tile([P, P], f32)
        nc.vector.memset(ones_f[:], 1.0)
        ones_bf = res.tile([P, P], bf)
        nc.vector.tensor_copy(ones_bf[:], ones_f[:])
        ones8 = res.tile([P, 2, P], f8)
        nc.vector.memset(ones8[:], 1.0)
        eps_t = res.tile([P, 1], f32)
        nc.vector.memset(eps_t[:], EPS)
        shift_t = res.tile([P, 1], f32)
        nc.vector.memset(shift_t[:], -EXPSHIFT)
        qgain = res.tile([P, G], f32)
        nc.gpsimd.dma_start(qgain[:], qgain_d.to_broadcast([P, G]))
        jTb = res.tile([HD, HD], bf)
        nc.sync.dma_start(jTb[:], jT_d[:])

        # ---- resident tensors ----
        kT = res.tile([P, S], bf)
        qTb = [res.tile([P, S], bf, tag=f"qT{h}", name=f"qT{h}") for h in range(G)]
        v_bf = res.tile([P, S // P, HD], bf)
        if PV_FP8:
            v8 = res.tile([P, S // P, HD], f8)
            v8r = res.tile([P, S // P, HD], f8)
        if PROJ_FP8:
            oT8 = res.tile([P, G, S], f8)
            oT8r = res.tile([P, G, S], f8)
        else:
            oTb = [res.tile([P, S], bf, tag=f"oT{h}", name=f"oT{h}")
                   for h in range(G)]

        # shared PSUM pools (8 banks):
        #   pair2: [P, 2*SL] f32 tiles (2 banks) x2  -> score pairs / psy halves
        #   one1:  [P, SL] f32 tiles (1 bank)  x4  -> qkv / ssq / qj / o / rs
        # phase-scoped pools below keep it simpler: each phase fits in 8.

        # ================= PHASE 1: QKV + RMSNorm + RoPE =================
        with ExitStack() as ph1:
            wpool = ph1.enter_context(tc.tile_pool(name="w", bufs=1))
            xtp = ph1.enter_context(tc.tile_pool(name="xt", bufs=2))
            tmp = ph1.enter_context(tc.tile_pool(name="p1tmp", bufs=3))
            ps1 = ph1.enter_context(tc.tile_pool(name="ps1", bufs=3, space="PSUM"))
            psv = ph1.enter_context(tc.tile_pool(name="psv", bufs=2, space="PSUM"))
            pssq = ph1.enter_context(tc.tile_pool(name="pssq", bufs=1, space="PSUM"))
            psj = ph1.enter_context(tc.tile_pool(name="psj", bufs=2, space="PSUM"))
            csp = ph1.enter_context(tc.tile_pool(name="cs", bufs=1))

            # startup DMA order: K weights + first x chunks first so the PE
            # starts ASAP; Q weights next; V weights before the V loop.
            wk8 = wpool.tile([P, DK, HD], wdt)
            nc.sync.dma_start(wk8[:], wk3[:])
            wq8 = wpool.tile([P, DK, G * HD], wdt)
            nc.sync.dma_start(wq8[:], wq3[:])
            NXC = 4  # x chunks for js=0
            CW = DK // NXC
            xt0 = xtp.tile([P, DK, SL], wdt, tag="x8", name="xt0")
            for c in range(NXC):
                nc.sync.dma_start(xt0[:, c * CW:(c + 1) * CW, :],
                                  x3[:, c * CW:(c + 1) * CW, 0:SL])
            if QKV_FP8:
                wk8r = wpool.tile([P, DK, HD], f8)
                nc.sync.dma_start(wk8r[:], wk3r_ := wk8r_d.rearrange("(o p) m -> p o m", p=P))
                xr0 = xtp.tile([P, DK, SL], f8, tag="x8r", name="xr0")
                for c in range(NXC):
                    nc.sync.dma_start(xr0[:, c * CW:(c + 1) * CW, :],
                                      x3r[:, c * CW:(c + 1) * CW, 0:SL])
                wq8r = wpool.tile([P, DK, G * HD], f8)
                nc.sync.dma_start(wq8r[:], wq8r_d.rearrange("(o p) m -> p o m", p=P))
            wv8 = wpool.tile([P, DK, HD], wdt)
            nc.sync.dma_start(wv8[:], wv3[:])
            if QKV_FP8:
                wv8r = wpool.tile([P, DK, HD], f8)
                nc.sync.dma_start(wv8r[:], wv8r_d.rearrange("(o p) m -> p o m", p=P))
            cos_sb = csp.tile([HD, S], bf)
            nc.sync.dma_start(cos_sb[:], cosT_d[:])
            sin_sb = csp.tile([HD, S], bf)
            nc.sync.dma_start(sin_sb[:], sinT_d[:])

            def proj_qkv(out_ps, whi, wlo, xhi, xlo, mslc):
                """Accumulate a QKV projection into out_ps.

                fp8: 3-term DoubleRow (w8 x8, w8r x8, w8 x8r), 24 instrs.
                bf16: plain 16-k-tile accumulation.
                """
                if QKV_FP8:
                    terms = [(whi, xhi), (wlo, xhi), (whi, xlo)]
                    n = len(terms) * NPAIR
                    j = 0
                    for wt, xt in terms:
                        for pr in range(NPAIR):
                            sl2 = slice(2 * pr, 2 * pr + 2)
                            nc.tensor.matmul(out_ps, wt[:, sl2, mslc],
                                             xt[:, sl2, :],
                                             start=(j == 0), stop=(j == n - 1),
                                             perf_mode=DR)
                            j += 1
                else:
                    for dk in range(DK):
                        nc.tensor.matmul(out_ps, whi[:, dk, mslc],
                                         xhi[:, dk, :],
                                         start=(dk == 0), stop=(dk == DK - 1))

            def proj_v(out_ps, whi, wlo, xhi, xlo, tslc):
                """V in natural [token, HD] layout: x chunk is stationary."""
                if QKV_FP8:
                    terms = [(whi, xhi), (wlo, xhi), (whi, xlo)]
                    n = len(terms) * NPAIR
                    j = 0
                    for wt, xt in terms:
                        for pr in range(NPAIR):
                            sl2 = slice(2 * pr, 2 * pr + 2)
                            nc.tensor.matmul(out_ps, xt[:, sl2, tslc],
                                             wt[:, sl2, :],
                                             start=(j == 0), stop=(j == n - 1),
                                             perf_mode=DR)
                            j += 1
                else:
                    for dk in range(DK):
                        nc.tensor.matmul(out_ps, xhi[:, dk, tslc],
                                         whi[:, dk, :],
                                         start=(dk == 0), stop=(dk == DK - 1))

            def norm_rope(src_ps, dst, js, gain_ap):
                """RMS-normalize (+gain) and RoPE a [128, SL] head block."""
                sq = tmp.tile([P, SL], bf, tag="sq")
                nc.scalar.square(sq[:], src_ps[:])
                ssq = pssq.tile([P, SL], f32, tag="ssq")
                nc.tensor.matmul(ssq[:], ones_bf[:], sq[:], start=True,
                                 stop=True)
                fb = tmp.tile([P, SL], bf, tag="fb")
                nc.scalar.activation(fb[:], ssq[:], AF.Sqrt,
                                     bias=eps_t[:], scale=1.0 / HD)
                with nc.allow_low_precision(reason="bf16 norm factor, 0.4% ok"):
                    nc.vector.reciprocal(fb[:], fb[:])
                if gain_ap is not None:
                    nc.vector.tensor_scalar_mul(fb[:], fb[:], gain_ap)
                qn = tmp.tile([P, SL], bf, tag="qn")
                nc.vector.tensor_mul(qn[:], src_ps[:], fb[:])
                qj = psj.tile([P, SL], f32, tag="qj")
                nc.tensor.matmul(qj[:], jTb[:], qn[:], start=True, stop=True)
                c = cos_sb[:, js * SL:(js + 1) * SL]
                s = sin_sb[:, js * SL:(js + 1) * SL]
                t1 = tmp.tile([P, SL], bf, tag="t1")
                t2 = tmp.tile([P, SL], bf, tag="t2")
                nc.vector.tensor_mul(t1[:], qn[:], c)
                nc.vector.tensor_mul(t2[:], qj[:], s)
                nc.vector.tensor_add(dst, t1[:], t2[:])

            for js in range(NSL):
                if js == 0:
                    xt, xr = xt0, (xr0 if QKV_FP8 else None)
                else:
                    xt = xtp.tile([P, DK, SL], wdt, tag="x8", name=f"xt{js}")
                    nc.sync.dma_start(xt[:], x3[:, :, js * SL:(js + 1) * SL])
                    if QKV_FP8:
                        xr = xtp.tile([P, DK, SL], f8, tag="x8r", name=f"xr{js}")
                        nc.sync.dma_start(xr[:], x3r[:, :, js * SL:(js + 1) * SL])
                    else:
                        xr = None

                # K
                k_ps = ps1.tile([P, SL], f32, tag="qkv")
                proj_qkv(k_ps[:], wk8, wk8r if QKV_FP8 else None, xt, xr,
                         slice(0, HD))
                norm_rope(k_ps, kT[:, js * SL:(js + 1) * SL], js, None)
                # V (natural layout, one 128-token tile at a time)
                for t in range(SL // P):
                    st = js * (SL // P) + t
                    v_ps = psv.tile([P, HD], f32, tag="v")
                    proj_v(v_ps[:], wv8, wv8r if QKV_FP8 else None, xt, xr,
                           slice(t * P, (t + 1) * P))
                    nc.scalar.copy(v_bf[:, st, :], v_ps[:])
                    if PV_FP8:
                        nc.scalar.copy(v8[:, st, :], v_ps[:])
                        nc.vector.tensor_sub(v8r[:, st, :], v_ps[:],
                                             v8[:, st, :])
                # Q heads
                for h in range(G):
                    q_ps = ps1.tile([P, SL], f32, tag="qkv")
                    proj_qkv(q_ps[:], wq8, wq8r if QKV_FP8 else None, xt, xr,
                             slice(h * HD, (h + 1) * HD))
                    norm_rope(q_ps, qTb[h][:, js * SL:(js + 1) * SL], js,
                              qgain[:, h:h + 1])

        # ================= PHASE 2: attention =====================
        with ExitStack() as ph2:
            ptp = ph2.enter_context(tc.tile_pool(name="pt", bufs=4))
            ptdp = ph2.enter_context(tc.tile_pool(name="ptd", bufs=4))
            bb = ph2.enter_context(tc.tile_pool(name="p2b", bufs=2))
            pssc = ph2.enter_context(tc.tile_pool(name="pssc", bufs=2,
                                                  space="PSUM"))
            pso = ph2.enter_context(tc.tile_pool(name="pso", bufs=2,
                                                 space="PSUM"))
            psrs = ph2.enter_context(tc.tile_pool(name="psrs", bufs=2,
                                                  space="PSUM"))

            for jq in range(NSL):
                for h in range(G):
                    qsl = slice(jq * SL, (jq + 1) * SL)
                    o_ps = pso.tile([P, SL], f32, tag="o")
                    rs_ps = psrs.tile([P, SL], f32, tag="rs")
                    started = False
                    # regular (full, causally-valid) key blocks, in pairs
                    for pi in range(2 * jq):
                        i0 = 2 * pi
                        sc2 = pssc.tile([P, 2, SL], f32, tag="sc")
                        for j in range(2):
                            nc.tensor.matmul(sc2[:, j, :],
                                             kT[:, (i0 + j) * P:(i0 + j + 1) * P],
                                             qTb[h][:, qsl],
                                             start=True, stop=True)
                        if PV_FP8:
                            pt8 = ptp.tile([P, 2, SL], f8, tag="pt8")
                            nc.scalar.activation(pt8[:], sc2[:], AF.Exp,
                                                 bias=shift_t[:])
                            nc.tensor.matmul(rs_ps[:], ones8[:], pt8[:],
                                             start=not started, stop=False,
                                             perf_mode=DR)
                            nc.tensor.matmul(o_ps[:], v8[:, i0:i0 + 2, :],
                                             pt8[:], start=not started,
                                             stop=False, perf_mode=DR)
                            nc.tensor.matmul(o_ps[:], v8r[:, i0:i0 + 2, :],
                                             pt8[:], start=False, stop=False,
                                             perf_mode=DR)
                        else:
                            ptb = ptp.tile([P, 2, SL], bf, tag="pt8")
                            nc.scalar.activation(ptb[:], sc2[:], AF.Exp,
                                                 bias=shift_t[:])
                            for j in range(2):
                                nc.tensor.matmul(rs_ps[:], ones_bf[:],
                                                 ptb[:, j, :],
                                                 start=not started and j == 0,
                                                 stop=False)
                                nc.tensor.matmul(o_ps[:],
                                                 v_bf[:, i0 + j, :],
                                                 ptb[:, j, :],
                                                 start=not started and j == 0,
                                                 stop=False)
                        started = True
                    # diagonal blocks (bf16, masked), two per sc pair tile
                    for dd in range(2):
                        sc2 = pssc.tile([P, 2, SL], f32, tag="sc")
                        for j in range(2):
                            delta = 2 * dd + j
                            i = 4 * jq + delta
                            lo = delta * P
                            nc.tensor.matmul(
                                sc2[:, j, lo:], kT[:, i * P:(i + 1) * P],
                                qTb[h][:, jq * SL + lo:(jq + 1) * SL],
                                start=True, stop=True)
                        for j in range(2):
                            delta = 2 * dd + j
                            i = 4 * jq + delta
                            lo = delta * P
                            span = SL - lo
                            ptd = ptdp.tile([P, SL], bf, tag="ptd")
                            nc.scalar.activation(ptd[:, :span],
                                                 sc2[:, j, lo:], AF.Exp,
                                                 bias=shift_t[:])
                            nc.gpsimd.affine_select(
                                out=ptd[:, :span], in_=ptd[:, :span],
                                compare_op=OP.is_ge, fill=0.0,
                                base=0, pattern=[[1, span]],
                                channel_multiplier=-1)
                            last = (dd == 1 and j == 1)
                            nc.tensor.matmul(rs_ps[:, lo:], ones_bf[:],
                                             ptd[:, :span],
                                             start=not started, stop=last)
                            nc.tensor.matmul(o_ps[:, lo:], v_bf[:, i, :],
                                             ptd[:, :span],
                                             start=not started, stop=last)
                            started = True
                    # normalize
                    rb = bb.tile([P, SL], bf, tag="rb")
                    with nc.allow_low_precision(reason="bf16 softmax denom"):
                        nc.vector.reciprocal(rb[:], rs_ps[:])
                    if PROJ_FP8:
                        tb = bb.tile([P, SL], bf, tag="tb")
                        nc.vector.tensor_mul(tb[:], o_ps[:], rb[:])
                        nc.scalar.copy(oT8[:, h, qsl], tb[:])
                        nc.vector.tensor_sub(oT8r[:, h, qsl], tb[:],
                                             oT8[:, h, qsl])
                    else:
                        nc.vector.tensor_mul(oTb[h][:, qsl], o_ps[:], rb[:])

        # ================= PHASE 3: output projection ====================
        with ExitStack() as ph3:
            wpp = ph3.enter_context(tc.tile_pool(name="wp", bufs=1))
            psy = ph3.enter_context(tc.tile_pool(name="psy", bufs=2,
                                                 space="PSUM"))
            yout = ph3.enter_context(tc.tile_pool(name="yout", bufs=2))

            wp_sb = wpp.tile([P, G, D], pdt)
            nc.sync.dma_start(wp_sb[:], wp_d.rearrange("(o p) m -> p o m", p=P))
            if PROJ_FP8:
                wpr_sb = wpp.tile([P, G, D], f8)
                nc.sync.dma_start(wpr_sb[:],
                                  wpr_d.rearrange("(o p) m -> p o m", p=P))

            for st in range(S // P):
                y_ps = psy.tile([P, D], f32, tag="y")
                y_sb = yout.tile([P, D], bf, tag="ysb")
                tsl = slice(st * P, (st + 1) * P)
                for os_ in range(D // SL):
                    osl = slice(os_ * SL, (os_ + 1) * SL)
                    if PROJ_FP8:
                        terms = [(oT8, wp_sb), (oT8r, wp_sb), (oT8, wpr_sb)]
                        n = 2 * len(terms)
                        j = 0
                        for ot, wt in terms:
                            for hp in range(2):
                                hsl = slice(2 * hp, 2 * hp + 2)
                                nc.tensor.matmul(
                                    y_ps[:, osl], ot[:, hsl, tsl],
                                    wt[:, hsl, osl],
                                    start=(j == 0), stop=(j == n - 1),
                                    perf_mode=DR)
                                j += 1
                    else:
                        for h in range(G):
                            nc.tensor.matmul(y_ps[:, osl],
                                             oTb[h][:, tsl],
                                             wp_sb[:, h, osl],
                                             start=(h == 0), stop=(h == G - 1))
                    if os_ % 2 == 0:
                        nc.scalar.copy(y_sb[:, osl], y_ps[:, osl])
                    else:
                        nc.vector.tensor_copy(y_sb[:, osl], y_ps[:, osl])
                nc.sync.dma_start(y_d[tsl, :], y_sb[:])

    nc.compile()
    return nc


def _rope_tables():
    """cos/sin tables in [HD, S] layout (half tables stacked twice) + J^T."""
    inv_freq = 1.0 / (ROPE_BASE ** (np.arange(0, HD, 2, dtype=np.float32) / HD))
    freqs = np.outer(np.arange(S, dtype=np.float32), inv_freq)
    c = np.cos(freqs).T.astype(np.float32)
    s = np.sin(freqs).T.astype(np.float32)
    cosf = np.concatenate([c, c], axis=0).copy()
    sinf = np.concatenate([s, s], axis=0).copy()
    half = HD // 2
    jT = np.zeros((HD, HD), np.float32)
    jT[np.arange(half) + half, np.arange(half)] = 1.0
    jT[np.arange(half), np.arange(half) + half] = -1.0
    return cosf, sinf, jT


def _split8(a):
    import ml_dtypes
    f8 = ml_dtypes.float8_e4m3
    hi = np.ascontiguousarray(a).astype(f8)
    lo = (a - hi.astype(np.float32)).astype(f8)
    return hi, lo


def make_in_maps(x, Wq, Wk, Wv, Wproj, q_gain):
    import ml_dtypes
    bf16 = ml_dtypes.bfloat16

    cosT, sinT, jT = _rope_tables()
    cosT = cosT.astype(bf16)
    sinT = sinT.astype(bf16)
    jT = jT.astype(bf16)
    x = np.asarray(x, np.float32)
    Wq = np.asarray(Wq, np.float32)
    Wk = np.asarray(Wk, np.float32)
    Wv = np.asarray(Wv, np.float32)
    WpT = np.ascontiguousarray(np.asarray(Wproj, np.float32).T)  # [in, out]
    q_gain = np.asarray(q_gain, np.float32)

    sw = SW if QKV_FP8 else 1.0
    xT = np.ascontiguousarray(np.transpose(x, (0, 2, 1)))  # [B, D, S]
    if QKV_FP8:
        x8 = [None] * B
        x8r = [None] * B
        for b in range(B):
            x8[b], x8r[b] = _split8(xT[b])

    in_maps = []
    for c in range(NCORES):
        b, g = divmod(c, KVH)
        sl_q = slice(g * G * HD, (g + 1) * G * HD)
        sl_kv = slice(g * HD, (g + 1) * HD)
        m = {
            "cosT": cosT, "sinT": sinT, "jT": jT,
            "qgain": (q_gain[g * G:(g + 1) * G] / np.sqrt(HD))
            .reshape(1, G).astype(np.float32),
        }
        wqT = np.ascontiguousarray(Wq[sl_q, :].T) * sw
        wkT = np.ascontiguousarray(Wk[sl_kv, :].T) * sw
        wvT = np.ascontiguousarray(Wv[sl_kv, :].T) * sw
        if QKV_FP8:
            m["x8"], m["x8r"] = x8[b], x8r[b]
            m["wq8"], m["wq8r"] = _split8(wqT)
            m["wk8"], m["wk8r"] = _split8(wkT)
            m["wv8"], m["wv8r"] = _split8(wvT)
        else:
            m["x8"] = xT[b].astype(bf16)
            m["wq8"] = wqT.astype(bf16)
            m["wk8"] = wkT.astype(bf16)
            m["wv8"] = wvT.astype(bf16)
        wp = np.ascontiguousarray(WpT[sl_q, :])
        if PROJ_FP8:
            m["wp"], m["wpr"] = _split8(wp * SW)
        else:
            m["wp"] = (wp / sw).astype(bf16)
        in_maps.append(m)
    return in_maps


def kernel(x, Wq, Wk, Wv, Wproj, q_gain):
    from concourse.bass_utils import run_bass_kernel_spmd

    if "nc" not in _CACHE:
        _CACHE["nc"] = _build_program()
    nc = _CACHE["nc"]

    in_maps = make_in_maps(x, Wq, Wk, Wv, Wproj, q_gain)
    res = run_bass_kernel_spmd(nc, in_maps, core_ids=list(range(NCORES)))
    _CACHE["last_results"] = res

    scale = (SW * SW) if PROJ_FP8 else 1.0
    y = np.zeros((B, S, D), dtype=np.float32)
    for c in range(NCORES):
        y[c // KVH] += np.asarray(res.results[c]["y"]).astype(np.float32)
    return (y / scale).astype(np.float32)


# revision 16
# speedup vs baseline: 1.2765x; 1.2765x over previous
"""Causal self-attention (GQA + QK-RMSNorm + RoPE + q_gain) on 8 Trainium2 cores.

Sharding: 8 cores = 2 (batch) x 4 (KV head group).  Core c handles batch
c//4 and KV group g=c%4 (Q heads 4g..4g+3); it computes its heads'
attention and a partial output projection; the host sums 4 partials per
batch.

Precision strategy (rel-err budget 2e-2):
- QKV projections: fp8e4m3 DoubleRow matmuls, 3-term residual scheme
  (w8 x8 + w8r x8 + w8 x8r) with weights pre-scaled x16 on the host so
  they sit in e4m3's normal range.  The x16 falls out of q/k via RMSNorm
  and is folded out of v via the Wproj scaling.
- Attention scores: bf16 (fp8 scores measured too lossy).
- Softmax: exp with a constant logit shift so e^logit fits e4m3.
  Off-diagonal ("regular") blocks quantize p to fp8 and use DoubleRow
  for the rowsum (paired ones) and PV (paired key blocks, v split
  hi+lo fp8); numerator and denominator use the SAME quantized p so the
  error acts as a small reweighting.  Diagonal blocks stay bf16 (their
  early tokens have tiny row maxima that would flush in fp8).
- Output projection bf16, y output bf16.

Scheduling: the PE executes in issue order, so dependent chains
(score->exp->PV, rmsnorm chains) are interleaved with independent matmul
groups: phase 1 staggers each head's norm chain behind the next head's
projection; phase 2 runs the output projection of the previous jq slice
between attention heads to cover the ACT-bound exp latency.
"""

import numpy as np

B, S, D = 2, 2048, 2048
H, KVH = 16, 4
HD = 128
G = H // KVH  # 4
NCORES = 8
ROPE_BASE = 10000.0
EPS = 1e-6

P = 128
SL = 512
NSL = S // SL        # 4
DK = D // P          # 16 contraction subtiles
NPAIR = DK // 2      # 8 DoubleRow pairs per term
SW = 16.0            # host weight scale for fp8 range
EXPSHIFT = 3.1       # logit shift so exp fits e4m3 (max logit ~8.15)

QKV_FP8 = True
PV_FP8 = True

_CACHE = {}


def _build_program():
    from contextlib import ExitStack

    import concourse.bass as bass
    import concourse.tile as tile
    from concourse import bacc, mybir

    f32 = mybir.dt.float32
    bf = mybir.dt.bfloat16
    f8 = mybir.dt.float8e4
    AF = mybir.ActivationFunctionType
    OP = mybir.AluOpType
    DR = mybir.MatmulPerfMode.DoubleRow

    nc = bacc.Bacc("TRN2", target_bir_lowering=False)

    wdt = f8 if QKV_FP8 else bf
    x8_d = nc.dram_tensor("x8", [D, S], wdt, kind="ExternalInput").ap()
    wq8_d = nc.dram_tensor("wq8", [D, G * HD], wdt, kind="ExternalInput").ap()
    wk8_d = nc.dram_tensor("wk8", [D, HD], wdt, kind="ExternalInput").ap()
    wv8_d = nc.dram_tensor("wv8", [D, HD], wdt, kind="ExternalInput").ap()
    if QKV_FP8:
        x8r_d = nc.dram_tensor("x8r", [D, S], f8, kind="ExternalInput").ap()
        wq8r_d = nc.dram_tensor("wq8r", [D, G * HD], f8, kind="ExternalInput").ap()
        wk8r_d = nc.dram_tensor("wk8r", [D, HD], f8, kind="ExternalInput").ap()
        wv8r_d = nc.dram_tensor("wv8r", [D, HD], f8, kind="ExternalInput").ap()
    wp_d = nc.dram_tensor("wp", [G * HD, D], bf, kind="ExternalInput").ap()
    cosT_d = nc.dram_tensor("cosT", [HD, S], bf, kind="ExternalInput").ap()
    sinT_d = nc.dram_tensor("sinT", [HD, S], bf, kind="ExternalInput").ap()
    jT_d = nc.dram_tensor("jT", [HD, HD], bf, kind="ExternalInput").ap()
    qgain_d = nc.dram_tensor("qgain", [1, G], f32, kind="ExternalInput").ap()
    y_d = nc.dram_tensor("y", [S, D], bf, kind="ExternalOutput").ap()

    x3 = x8_d.rearrange("(o p) s -> p o s", p=P)
    if QKV_FP8:
        x3r = x8r_d.rearrange("(o p) s -> p o s", p=P)

    with tile.TileContext(nc) as tc, ExitStack() as top:
        res = top.enter_context(tc.tile_pool(name="resident", bufs=1))

        # ---- small constants ----
        ones_f = res.tile([P, P], f32)
        nc.vector.memset(ones_f[:], 1.0)
        ones_bf = res.tile([P, P], bf)
        nc.vector.tensor_copy(ones_bf[:], ones_f[:])
        ones8 = res.tile([P, 2, P], f8)
        nc.vector.memset(ones8[:], 1.0)
        eps_t = res.tile([P, 1], f32)
        nc.vector.memset(eps_t[:], EPS)
        shift_t = res.tile([P, 1], f32)
        nc.vector.memset(shift_t[:], -EXPSHIFT)
        qgain = res.tile([P, G], f32)
        nc.gpsimd.dma_start(qgain[:], qgain_d.to_broadcast([P, G]))
        jTb = res.tile([HD, HD], bf)  # DMA'd in phase 1 after x/w loads

        # ---- resident tensors ----
        kT = res.tile([P, S], bf)
        qTb = [res.tile([P, S], bf, tag=f"qT{h}", name=f"qT{h}") for h in range(G)]
        v_bf = res.tile([P, S // P, HD], bf)
        if PV_FP8:
            v8 = res.tile([P, S // P, HD], f8)
            v8r = res.tile([P, S // P, HD], f8)
        oTb = [res.tile([P, S], bf, tag=f"oT{h}", name=f"oT{h}") for h in range(G)]
        wp_sb = res.tile([P, G, D], bf)

        # ================= PHASE 1: QKV + RMSNorm + RoPE =================
        with ExitStack() as ph1:
            wpool = ph1.enter_context(tc.tile_pool(name="w", bufs=1))
            xtp = ph1.enter_context(tc.tile_pool(name="xt", bufs=2))
            tmp = ph1.enter_context(tc.tile_pool(name="p1tmp", bufs=3))
            ps1 = ph1.enter_context(tc.tile_pool(name="ps1", bufs=3, space="PSUM"))
            psv = ph1.enter_context(tc.tile_pool(name="psv", bufs=2, space="PSUM"))
            pssq = ph1.enter_context(tc.tile_pool(name="pssq", bufs=2, space="PSUM"))
            psj = ph1.enter_context(tc.tile_pool(name="psj", bufs=1, space="PSUM"))
            csp = ph1.enter_context(tc.tile_pool(name="cs", bufs=1))

            # startup DMA order: K weights and the first x chunks first so
            # the PE starts ASAP.
            wk8 = wpool.tile([P, DK, HD], wdt)
            nc.sync.dma_start(wk8[:], wk8_d.rearrange("(o p) m -> p o m", p=P))
            NXC = 4
            CW = DK // NXC
            xt0 = xtp.tile([P, DK, SL], wdt, tag="x8", name="xt0")
            nc.sync.dma_start(xt0[:, 0:CW, :], x3[:, 0:CW, 0:SL])
            if QKV_FP8:
                wk8r = wpool.tile([P, DK, HD], f8)
                nc.sync.dma_start(wk8r[:],
                                  wk8r_d.rearrange("(o p) m -> p o m", p=P))
            for c in range(1, NXC):
                nc.sync.dma_start(xt0[:, c * CW:(c + 1) * CW, :],
                                  x3[:, c * CW:(c + 1) * CW, 0:SL])
            if QKV_FP8:
                xr0 = xtp.tile([P, DK, SL], f8, tag="x8r", name="xr0")
                for c in range(NXC):
                    nc.sync.dma_start(xr0[:, c * CW:(c + 1) * CW, :],
                                      x3r[:, c * CW:(c + 1) * CW, 0:SL])
                wq8r = wpool.tile([P, DK, G * HD], f8)
            else:
                xr0 = None
            wq8 = wpool.tile([P, DK, G * HD], wdt)
            nc.sync.dma_start(wq8[:], wq8_d.rearrange("(o p) m -> p o m", p=P))
            if QKV_FP8:
                nc.sync.dma_start(wq8r[:],
                                  wq8r_d.rearrange("(o p) m -> p o m", p=P))
            wv8 = wpool.tile([P, DK, HD], wdt)
            nc.sync.dma_start(wv8[:], wv8_d.rearrange("(o p) m -> p o m", p=P))
            if QKV_FP8:
                wv8r = wpool.tile([P, DK, HD], f8)
                nc.sync.dma_start(wv8r[:],
                                  wv8r_d.rearrange("(o p) m -> p o m", p=P))
            cos_sb = csp.tile([HD, S], bf)
            nc.sync.dma_start(cos_sb[:], cosT_d[:])
            sin_sb = csp.tile([HD, S], bf)
            nc.sync.dma_start(sin_sb[:], sinT_d[:])
            nc.sync.dma_start(jTb[:], jT_d[:])

            def proj_qkv(out_ps, whi, wlo, xhi, xlo, mslc):
                if QKV_FP8:
                    terms = [(whi, xhi), (wlo, xhi), (whi, xlo)]
                    n = len(terms) * NPAIR
                    j = 0
                    for wt, xt_ in terms:
                        for pr in range(NPAIR):
                            sl2 = slice(2 * pr, 2 * pr + 2)
                            nc.tensor.matmul(out_ps, wt[:, sl2, mslc],
                                             xt_[:, sl2, :],
                                             start=(j == 0), stop=(j == n - 1),
                                             perf_mode=DR)
                            j += 1
                else:
                    for dk in range(DK):
                        nc.tensor.matmul(out_ps, whi[:, dk, mslc],
                                         xhi[:, dk, :],
                                         start=(dk == 0), stop=(dk == DK - 1))

            def proj_v(out_ps, xhi, xlo, tslc):
                if QKV_FP8:
                    terms = [(wv8, xhi), (wv8r, xhi), (wv8, xlo)]
                    n = len(terms) * NPAIR
                    j = 0
                    for wt, xt_ in terms:
                        for pr in range(NPAIR):
                            sl2 = slice(2 * pr, 2 * pr + 2)
                            nc.tensor.matmul(out_ps, xt_[:, sl2, tslc],
                                             wt[:, sl2, :],
                                             start=(j == 0), stop=(j == n - 1),
                                             perf_mode=DR)
                            j += 1
                else:
                    for dk in range(DK):
                        nc.tensor.matmul(out_ps, xhi[:, dk, tslc],
                                         wv8[:, dk, :],
                                         start=(dk == 0), stop=(dk == DK - 1))

            # rmsnorm+rope split into stages so each head's dependent chain
            # is issued behind the next head's (independent) projection.
            def stage_sq(src_ps):
                sq = tmp.tile([P, SL], bf, tag="sq")
                nc.scalar.square(sq[:], src_ps[:])
                return sq

            def stage_ssq(sq):
                ssq = pssq.tile([P, SL], f32, tag="ssq")
                nc.tensor.matmul(ssq[:], ones_bf[:], sq[:], start=True,
                                 stop=True)
                return ssq

            def stage_rope(src_ps, ssq, dst, js, gain_ap):
                fb = tmp.tile([P, SL], f32, tag="fb")
                nc.scalar.activation(fb[:], ssq[:], AF.Sqrt,
                                     bias=eps_t[:], scale=1.0 / HD)
                nc.vector.reciprocal(fb[:], fb[:])
                if gain_ap is not None:
                    nc.vector.tensor_scalar_mul(fb[:], fb[:], gain_ap)
                qn = tmp.tile([P, SL], bf, tag="qn")
                nc.vector.tensor_mul(qn[:], src_ps[:], fb[:])
                qj = psj.tile([P, SL], f32, tag="qj")
                nc.tensor.matmul(qj[:], jTb[:], qn[:], start=True, stop=True)
                c = cos_sb[:, js * SL:(js + 1) * SL]
                s = sin_sb[:, js * SL:(js + 1) * SL]
                t1 = tmp.tile([P, SL], f32, tag="t1")
                t2 = tmp.tile([P, SL], f32, tag="t2")
                # t1/add run on the (otherwise idle) Pool engine to keep the
                # DVE off the phase-1 critical path
                nc.gpsimd.tensor_mul(t1[:], qn[:], c)
                nc.vector.tensor_mul(t2[:], qj[:], s)
                nc.gpsimd.tensor_add(dst, t1[:], t2[:])

            for js in range(NSL):
                if js == 0:
                    xt, xr = xt0, xr0
                else:
                    xt = xtp.tile([P, DK, SL], wdt, tag="x8", name=f"xt{js}")
                    nc.sync.dma_start(xt[:], x3[:, :, js * SL:(js + 1) * SL])
                    if QKV_FP8:
                        xr = xtp.tile([P, DK, SL], f8, tag="x8r",
                                      name=f"xr{js}")
                        nc.sync.dma_start(xr[:],
                                          x3r[:, :, js * SL:(js + 1) * SL])
                    else:
                        xr = None
                    if js == 1:
                        # prefetch the projection weights (phase 2 filler
                        # needs them); deferred past the startup-critical
                        # x/w loads
                        nc.sync.dma_start(
                            wp_sb[:], wp_d.rearrange("(o p) m -> p o m", p=P))

                jsl = slice(js * SL, (js + 1) * SL)
                # staged emission: each chain step sits behind independent
                # PE work from the following projection group.
                k_ps = ps1.tile([P, SL], f32, tag="qkv")
                proj_qkv(k_ps[:], wk8, wk8r if QKV_FP8 else None, xt, xr,
                         slice(0, HD))
                sq_k = stage_sq(k_ps)

                q_ps = [None] * G
                sq_q = [None] * G
                ssq_q = [None] * G
                chains = []  # (src_ps, ssq, dst, gain) pending stage_rope
                ssq_k = None
                for h in range(G):
                    q_ps[h] = ps1.tile([P, SL], f32, tag="qkv",
                                       name=f"q{h}_ps")
                    proj_qkv(q_ps[h][:], wq8, wq8r if QKV_FP8 else None,
                             xt, xr, slice(h * HD, (h + 1) * HD))
                    if h == 0:
                        ssq_k = stage_ssq(sq_k)
                        sq_q[0] = stage_sq(q_ps[0])
                    else:
                        if h == 1:
                            stage_rope(k_ps, ssq_k, kT[:, jsl], js, None)
                        else:
                            stage_rope(q_ps[h - 2], ssq_q[h - 2],
                                       qTb[h - 2][:, jsl], js,
                                       qgain[:, h - 2:h - 1])
                        ssq_q[h - 1] = stage_ssq(sq_q[h - 1])
                        sq_q[h] = stage_sq(q_ps[h])
                # V tiles interleave with the remaining two q chains
                for t in range(SL // P):
                    st = js * (SL // P) + t
                    if t == 0:
                        stage_rope(q_ps[G - 2], ssq_q[G - 2],
                                   qTb[G - 2][:, jsl], js,
                                   qgain[:, G - 2:G - 1])
                        ssq_q[G - 1] = stage_ssq(sq_q[G - 1])
                    v_ps = psv.tile([P, HD], f32, tag="v")
                    proj_v(v_ps[:], xt, xr, slice(t * P, (t + 1) * P))
                    if t == 2:
                        stage_rope(q_ps[G - 1], ssq_q[G - 1],
                                   qTb[G - 1][:, jsl], js,
                                   qgain[:, G - 1:G])
                    nc.scalar.copy(v_bf[:, st, :], v_ps[:])
                    if PV_FP8:
                        nc.scalar.copy(v8[:, st, :], v_ps[:])
                        nc.vector.tensor_sub(v8r[:, st, :], v_ps[:],
                                             v8[:, st, :])

        # ============ PHASE 2: attention + interleaved projection ========
        with ExitStack() as ph2:
            ptp = ph2.enter_context(tc.tile_pool(name="pt", bufs=4))
            ptdp = ph2.enter_context(tc.tile_pool(name="ptd", bufs=4))
            bb = ph2.enter_context(tc.tile_pool(name="p2b", bufs=2))
            yout = ph2.enter_context(tc.tile_pool(name="yout", bufs=2))
            pssc = ph2.enter_context(tc.tile_pool(name="pssc", bufs=2,
                                                  space="PSUM"))
            pso = ph2.enter_context(tc.tile_pool(name="pso", bufs=2,
                                                 space="PSUM"))
            psrs = ph2.enter_context(tc.tile_pool(name="psrs", bufs=2,
                                                  space="PSUM"))

            def emit_proj(st):
                """Output projection for one 128-token tile (dense PE work,
                no ACT dependency -- used to fill attention's exp latency).
                PSUM comes from the "rs"-tagged slots, which free quickly
                (right after each head's reciprocal), so proj never inherits
                the exp-paced "sc" slot rotation."""
                tsl = slice(st * P, (st + 1) * P)
                y_sb = yout.tile([P, D], bf, tag="ysb", name=f"y{st}")
                for os_ in range(D // SL):
                    osl = slice(os_ * SL, (os_ + 1) * SL)
                    y1 = psrs.tile([P, SL], f32, tag="rs",
                                   name=f"yps{st}_{os_}")
                    for h in range(G):
                        nc.tensor.matmul(y1[:], oTb[h][:, tsl],
                                         wp_sb[:, h, osl],
                                         start=(h == 0), stop=(h == G - 1))
                    if os_ % 2 == 0:
                        nc.scalar.copy(y_sb[:, osl], y1[:])
                    else:
                        nc.vector.tensor_copy(y_sb[:, osl], y1[:])
                nc.sync.dma_start(y_d[tsl, :], y_sb[:])

            proj_queue = []
            pdt_ = f8 if PV_FP8 else bf

            class Head:
                """Per-head PSUM accumulation state."""

                def __init__(self, h, jq):
                    self.h = h
                    self.jq = jq
                    self.qsl = slice(jq * SL, (jq + 1) * SL)
                    self.o_ps = pso.tile([P, SL], f32, tag="o",
                                         name=f"o{jq}_{h}")
                    self.rs_ps = psrs.tile([P, SL], f32, tag="rs",
                                           name=f"rs{jq}_{h}")
                    self.started = False
                    self.left = 2 * jq + 2  # units to consume

                def produce_reg(self, pi):
                    i0 = 2 * pi
                    sc2 = pssc.tile([P, 2, SL], f32, tag="sc")
                    for j in range(2):
                        nc.tensor.matmul(sc2[:, j, :],
                                         kT[:, (i0 + j) * P:(i0 + j + 1) * P],
                                         qTb[self.h][:, self.qsl],
                                         start=True, stop=True)
                    pt8 = ptp.tile([P, 2, SL], pdt_, tag="pt8")
                    nc.scalar.activation(pt8[:], sc2[:], AF.Exp,
                                         bias=shift_t[:])
                    return ("reg", pt8, i0)

                def produce_diag(self, dd):
                    jq = self.jq
                    sc2 = pssc.tile([P, 2, SL], f32, tag="sc")
                    for j in range(2):
                        lo = (2 * dd + j) * P
                        nc.tensor.matmul(
                            sc2[:, j, lo:],
                            kT[:, (4 * jq + 2 * dd + j) * P:
                               (4 * jq + 2 * dd + j + 1) * P],
                            qTb[self.h][:, jq * SL + lo:(jq + 1) * SL],
                            start=True, stop=True)
                    blocks = []
                    for j in range(2):
                        delta = 2 * dd + j
                        lo = delta * P
                        span = SL - lo
                        ptd = ptdp.tile([P, SL], bf, tag="ptd")
                        nc.scalar.activation(ptd[:, :span], sc2[:, j, lo:],
                                             AF.Exp, bias=shift_t[:])
                        nc.gpsimd.affine_select(
                            out=ptd[:, :span], in_=ptd[:, :span],
                            compare_op=OP.is_ge, fill=0.0,
                            base=0, pattern=[[1, span]],
                            channel_multiplier=-1)
                        blocks.append((ptd, 4 * jq + delta, lo, span))
                    return ("diag", blocks)

                def consume(self, unit):
                    st0 = not self.started
                    self.started = True
                    self.left -= 1
                    last_u = self.left == 0
                    if unit[0] == "reg":
                        _, pt8, i0 = unit
                        if PV_FP8:
                            nc.tensor.matmul(self.rs_ps[:], ones8[:], pt8[:],
                                             start=st0, stop=False,
                                             perf_mode=DR)
                            nc.tensor.matmul(self.o_ps[:], v8[:, i0:i0 + 2, :],
                                             pt8[:], start=st0, stop=False,
                                             perf_mode=DR)
                            nc.tensor.matmul(self.o_ps[:],
                                             v8r[:, i0:i0 + 2, :],
                                             pt8[:], start=False, stop=False,
                                             perf_mode=DR)
                        else:
                            for j in range(2):
                                nc.tensor.matmul(self.rs_ps[:], ones_bf[:],
                                                 pt8[:, j, :],
                                                 start=st0 and j == 0,
                                                 stop=False)
                                nc.tensor.matmul(self.o_ps[:],
                                                 v_bf[:, i0 + j, :],
                                                 pt8[:, j, :],
                                                 start=st0 and j == 0,
                                                 stop=False)
                    else:
                        for n_, (ptd, i, lo, span) in enumerate(unit[1]):
                            last = last_u and n_ == len(unit[1]) - 1
                            nc.tensor.matmul(self.rs_ps[:, lo:], ones_bf[:],
                                             ptd[:, :span],
                                             start=st0 and n_ == 0, stop=last)
                            nc.tensor.matmul(self.o_ps[:, lo:],
                                             v_bf[:, i, :],
                                             ptd[:, :span],
                                             start=st0 and n_ == 0, stop=last)
                    if last_u:
                        rb = bb.tile([P, SL], f32, tag="rb")
                        nc.vector.reciprocal(rb[:], self.rs_ps[:])
                        nc.vector.tensor_mul(oTb[self.h][:, self.qsl],
                                             self.o_ps[:], rb[:])

            # Cross-head pipelined emission: producers (scores+exp, ACT
            # bound) run 2 units ahead of consumers (DR matmuls), so the PE
            # never waits on an exp.  Output-projection tiles of the
            # previous jq slice are emitted at head boundaries as dense PE
            # filler.  jq=0 (diagonal-only) runs last, fed by jq=3's tiles.
            LOOKAHEAD = 2
            for jq in (1, 2, 3, 0):
                pending = []
                nfill = 2 if jq == 0 else 1
                for h in range(G):
                    head = Head(h, jq)
                    units = ([("reg", pi) for pi in range(2 * jq)]
                             + [("diag", dd) for dd in range(2)])
                    for kind, arg in units:
                        if kind == "reg":
                            u = head.produce_reg(arg)
                        else:
                            u = head.produce_diag(arg)
                        pending.append((head, u))
                        while len(pending) > LOOKAHEAD:
                            hd, uu = pending.pop(0)
                            hd.consume(uu)
                    for _ in range(nfill):
                        if proj_queue:
                            emit_proj(proj_queue.pop(0))
                while pending:
                    hd, uu = pending.pop(0)
                    hd.consume(uu)
                proj_queue.extend(range(jq * 4, jq * 4 + 4))

            for st in proj_queue:
                emit_proj(st)

    nc.compile()
    return nc


def _rope_tables():
    """cos/sin tables in [HD, S] layout (half tables stacked twice) + J^T."""
    inv_freq = 1.0 / (ROPE_BASE ** (np.arange(0, HD, 2, dtype=np.float32) / HD))
    freqs = np.outer(np.arange(S, dtype=np.float32), inv_freq)
    c = np.cos(freqs).T.astype(np.float32)
    s = np.sin(freqs).T.astype(np.float32)
    cosf = np.concatenate([c, c], axis=0).copy()
    sinf = np.concatenate([s, s], axis=0).copy()
    half = HD // 2
    jT = np.zeros((HD, HD), np.float32)
    jT[np.arange(half) + half, np.arange(half)] = 1.0
    jT[np.arange(half), np.arange(half) + half] = -1.0
    return cosf, sinf, jT


def _split8(a):
    import ml_dtypes
    f8 = ml_dtypes.float8_e4m3
    hi = np.ascontiguousarray(a).astype(f8)
    lo = (a - hi.astype(np.float32)).astype(f8)
    return hi, lo


def make_in_maps(x, Wq, Wk, Wv, Wproj, q_gain):
    import ml_dtypes
    bf16 = ml_dtypes.bfloat16

    cosT, sinT, jT = _rope_tables()
    cosT = cosT.astype(bf16)
    sinT = sinT.astype(bf16)
    jT = jT.astype(bf16)
    x = np.asarray(x, np.float32)
    Wq = np.asarray(Wq, np.float32)
    Wk = np.asarray(Wk, np.float32)
    Wv = np.asarray(Wv, np.float32)
    WpT = np.ascontiguousarray(np.asarray(Wproj, np.float32).T)  # [in, out]
    q_gain = np.asarray(q_gain, np.float32)

    sw = SW if QKV_FP8 else 1.0
    xT = np.ascontiguousarray(np.transpose(x, (0, 2, 1)))  # [B, D, S]
    if QKV_FP8:
        x8 = [None] * B
        x8r = [None] * B
        for b in range(B):
            x8[b], x8r[b] = _split8(xT[b])

    in_maps = []
    for c in range(NCORES):
        b, g = divmod(c, KVH)
        sl_q = slice(g * G * HD, (g + 1) * G * HD)
        sl_kv = slice(g * HD, (g + 1) * HD)
        m = {
            "cosT": cosT, "sinT": sinT, "jT": jT,
            "qgain": (q_gain[g * G:(g + 1) * G] / np.sqrt(HD))
            .reshape(1, G).astype(np.float32),
        }
        wqT = np.ascontiguousarray(Wq[sl_q, :].T) * sw
        wkT = np.ascontiguousarray(Wk[sl_kv, :].T) * sw
        wvT = np.ascontiguousarray(Wv[sl_kv, :].T) * sw
        if QKV_FP8:
            m["x8"], m["x8r"] = x8[b], x8r[b]
            m["wq8"], m["wq8r"] = _split8(wqT)
            m["wk8"], m["wk8r"] = _split8(wkT)
            m["wv8"], m["wv8r"] = _split8(wvT)
        else:
            m["x8"] = xT[b].astype(bf16)
            m["wq8"] = wqT.astype(bf16)
            m["wk8"] = wkT.astype(bf16)
            m["wv8"] = wvT.astype(bf16)
        m["wp"] = (np.ascontiguousarray(WpT[sl_q, :]) / sw).astype(bf16)
        in_maps.append(m)
    return in_maps


def kernel(x, Wq, Wk, Wv, Wproj, q_gain):
    from concourse.bass_utils import run_bass_kernel_spmd

    if "nc" not in _CACHE:
        _CACHE["nc"] = _build_program()
    nc = _CACHE["nc"]

    in_maps = make_in_maps(x, Wq, Wk, Wv, Wproj, q_gain)
    res = run_bass_kernel_spmd(nc, in_maps, core_ids=list(range(NCORES)))
    _CACHE["last_results"] = res

    y = np.zeros((B, S, D), dtype=np.float32)
    for c in range(NCORES):
        y[c // KVH] += np.asarray(res.results[c]["y"]).astype(np.float32)
    return y
